# revision 16
# baseline (speedup 1.0000x reference)
"""Trainium2 Bass kernel for nn_BidirLSTMModel (2-layer bidirectional LSTM + vocab head).

Sharding: each LSTM layer runs as one 8-core SPMD launch sharded by
(direction x batch-quarter): cores 0-3 = forward cells on batch quarters 0-3,
cores 4-7 = backward cells (inputs time-reversed on the host, so every core
runs the identical forward-scan program) => 64 batch rows per core. The output
head is a third launch sharded by vocab. The host moves the small intermediate
tensors between launches (free: only device launches count toward HW time).

Device layout is GATE-MAJOR throughout: weights are the 128x128 stationary
operands (FWL keeps LDWEIGHTS fully hidden; measured 38 ns per N=64 matmul),
h/c/gates all live as [feature-part, chunk, batch] so the recurrence needs no
transpose anywhere. Per 2-step PSUM group and gate g (psum tile [128,2,4,64]):
  - one [bias; +/-BIG*(1-mask)] rank-2 matmul per (gate, feature-chunk) seeds
    bias + length-mask penalties (saturating i/f/o sigmoids reproduces
    dynamic_rnn's masking exactly, as in the original reference trick),
  - xg matmuls stream 2 timesteps per instruction (N=128, 89% PE eff),
  - 4 recurrent Wh matmuls per (step, gate, fchunk) at N=64,
  - scalar engine applies tanh/sigmoid straight off PSUM slices,
  - vector engine does the c/h elementwise chain; h feeds the next step's
    matmuls directly as the moving operand.
"""

import numpy as np
import ml_dtypes

import concourse.bass as bass
import concourse.mybir as mybir
import concourse.tile as tile
from concourse.bass_utils import run_bass_kernel_spmd


def _split_sync_waits(nc, max_waits=1):
    """This walrus build accepts at most one sync-wait per instruction; hoist
    extra waits onto same-engine NoOps placed immediately before (same queue,
    program order => identical wait-all semantics)."""
    n = 0
    for f in nc.m.functions:
        for bb in f.blocks:
            out = []
            for ins in bb.instructions:
                si = ins.sync_info
                if si is not None and si.on_wait and len(si.on_wait) > max_waits:
                    waits = list(si.on_wait)
                    for w in waits[:-max_waits]:
                        nop = mybir.InstNoOp(name=f"{ins.name}-ws{n}", ins=[], outs=[])
                        n += 1
                        nop.engine = ins.engine
                        nop.sync_info = mybir.SyncInfo(on_wait=[w], on_update=[])
                        out.append(nop)
                    si.on_wait = waits[-max_waits:]
                out.append(ins)
            bb.instructions[:] = out


BF16 = mybir.dt.bfloat16
F32 = mybir.dt.float32
NPBF = ml_dtypes.bfloat16

V, E, D, B, T = 50000, 128, 512, 256, 128
NC = 8
BSH = B // (NC // 2)   # 64 batch rows per core
BIG = 30.0
VSH = 6272             # padded vocab shard (49*128); 8*6250 = 50000
VTILES = VSH // 128    # 49
TB = 16                # xt streaming block (timesteps)

ACT = mybir.ActivationFunctionType
OP = mybir.AluOpType

_cache = {}


def _layer_program(nc, tc, pools, KC, xt_in, m2_in, wx_in, wb_in, wh_in,
                   y_out, states_out):
    """One direction of one layer, 64 batch rows, gate-major."""
    (cpool, xs, sv, cst, psp) = pools

    wh_sb = cpool.tile([128, 4, 16, 128], BF16)
    nc.sync.dma_start(wh_sb[:], wh_in[:])
    wx_sb = cpool.tile([128, KC, 16, 128], BF16)
    nc.sync.dma_start(wx_sb[:], wx_in[:])
    wb_sb = cpool.tile([2, 16, 128], BF16)
    nc.sync.dma_start(wb_sb[:], wb_in[:])
    m2_sb = cpool.tile([2, T, BSH], BF16)
    nc.sync.dma_start(m2_sb[:], m2_in[:])

    # xt: layer0's fits in SBUF whole (2 MB) — load it all so the scan never
    # waits on input DMA; layer1's (16 MB) streams in TB-step blocks.
    LEAD = 2
    xblks = {}

    if KC == 1:
        xall = cpool.tile([128, 1, T, BSH], BF16)
        for qq in range(4):
            nc.sync.dma_start(xall[:, :, qq * (T // 4):(qq + 1) * (T // 4), :],
                              xt_in[:, :, qq * (T // 4):(qq + 1) * (T // 4), :])

        def load_xblk(bi):
            if bi * TB < T:
                xblks[bi] = xall[:, :, bi * TB:(bi + 1) * TB, :]
    else:
        def load_xblk(bi):
            if bi >= T // TB:
                return
            tl = xs.tile([128, KC, TB, BSH], BF16, tag="xb", name="xb")
            for k in range(KC):
                nc.sync.dma_start(tl[:, k], xt_in[:, k, bi * TB:(bi + 1) * TB, :])
            xblks[bi] = tl

    for bi in range(LEAD):
        load_xblk(bi)

    h = sv.tile([128, 4, BSH], BF16, tag="h")
    nc.vector.memset(h[:], 0.0)
    c = cst.tile([128, 4, BSH], F32, tag="c")
    nc.vector.memset(c[:], 0.0)

    GS = 4  # steps per PSUM group
    for g in range(T // GS):
        s0 = GS * g
        if s0 % TB == 0:
            load_xblk(s0 // TB + LEAD)
        xb = xblks[s0 // TB]
        xo = s0 % TB

        # psum gate tiles: [128, fc, step, batch] so every matmul dst is a
        # contiguous slice (matmul outputs cannot be strided APs). Each tile
        # is 2 PSUM banks (single-buffered; 4 gates x 2 banks = all 8).
        # start=True marks one 2KB bank pending-zero, so the first writer of
        # each bank half (the fc==0 / fc==2 seed matmul) carries it.
        # Gate emission order j,i,f,o: the previous group's tanh_j/sig_i
        # drain early, so this group's j/i seed+xg matmuls fill the PE while
        # the previous group's o-activation and c/h chain finish.
        pt = [psp.tile([128, 4, GS, BSH], F32, tag=f"g{gt}", name=f"g{gt}")
              for gt in range(4)]
        for gt in range(4):
            # bias + mask-penalty seed matmuls
            for fc in range(4):
                nc.tensor.matmul(pt[gt][:, fc],
                                 wb_sb[:, 4 * gt + fc, :],
                                 m2_sb[:, s0:s0 + GS, :],
                                 start=(fc % 2 == 0), stop=False,
                                 skip_group_check=True)
            # input projection, GS steps per matmul
            for fc in range(4):
                for k in range(KC):
                    nc.tensor.matmul(pt[gt][:, fc],
                                     wx_sb[:, k, 4 * gt + fc, :],
                                     xb[:, k, xo:xo + GS, :],
                                     start=False, stop=False,
                                     skip_group_check=True)
        # GS sequential steps
        for ss in range(GS):
            s = s0 + ss
            sig = {}
            u = v = c2 = tcn = None
            for gt in range(4):
                for fc in range(4):
                    for kk in range(4):
                        nc.tensor.matmul(pt[gt][:, fc, ss, :],
                                         wh_sb[:, kk, 4 * gt + fc, :],
                                         h[:, kk, :],
                                         start=False, stop=(kk == 3),
                                         skip_group_check=True)
                g_t = sv.tile([128, 4, BSH], BF16, tag=f"s{gt}", name=f"s{gt}")
                nc.scalar.activation(g_t[:], pt[gt][:, :, ss, :],
                                     ACT.Tanh if gt == 0 else ACT.Sigmoid)
                sig[gt] = g_t
                if gt == 1:
                    u = sv.tile([128, 4, BSH], BF16, tag="u")
                    nc.vector.tensor_tensor(u[:], sig[1][:], sig[0][:], OP.mult)
                elif gt == 2:
                    v = sv.tile([128, 4, BSH], F32, tag="v")
                    nc.vector.tensor_tensor(v[:], sig[2][:], c[:], OP.mult)
                    c2 = cst.tile([128, 4, BSH], F32, tag="c")
                    nc.vector.tensor_tensor(c2[:], v[:], u[:], OP.add)
                    tcn = sv.tile([128, 4, BSH], BF16, tag="tcn")
                    nc.scalar.activation(tcn[:], c2[:], ACT.Tanh)
            h = sv.tile([128, 4, BSH], BF16, tag="h")
            nc.vector.tensor_tensor(h[:], sig[3][:], tcn[:], OP.mult)
            nc.sync.dma_start(y_out[s], h[:])
            c = c2

    nc.sync.dma_start(states_out[:], c[:])


def get_layer_nc(layer):
    key = f"layer{layer}"
    if key in _cache:
        return _cache[key]
    KC = 1 if layer == 0 else 8
    nc = bass.Bass()
    xt_in = nc.declare_dram_parameter("xt", [128, KC, T, BSH], BF16, isOutput=False)
    m2_in = nc.declare_dram_parameter("m2", [2, T, BSH], BF16, isOutput=False)
    wx_in = nc.declare_dram_parameter("wx", [128, KC, 16, 128], BF16, isOutput=False)
    wb_in = nc.declare_dram_parameter("wb", [2, 16, 128], BF16, isOutput=False)
    wh_in = nc.declare_dram_parameter("wh", [128, 4, 16, 128], BF16, isOutput=False)
    y_out = nc.declare_dram_parameter("y", [T, 128, 4, BSH], BF16, isOutput=True)
    states_out = nc.declare_dram_parameter("states", [128, 4, BSH], F32, isOutput=True)

    with tile.TileContext(nc) as tc:
        with (
            tc.tile_pool(name="const", bufs=1) as cpool,
            tc.tile_pool(name="xs", bufs=3) as xs,
            tc.tile_pool(name="sv", bufs=3) as sv,
            tc.tile_pool(name="cst", bufs=2) as cst,
            tc.tile_pool(name="psum", bufs=1, space="PSUM") as psp,
        ):
            pools = (cpool, xs, sv, cst, psp)
            _layer_program(nc, tc, pools, KC, xt_in, m2_in, wx_in, wb_in,
                           wh_in, y_out, states_out)
    _split_sync_waits(nc)
    _cache[key] = nc
    return nc


def get_head_nc():
    if "head" in _cache:
        return _cache["head"]
    nc = bass.Bass()
    stt_in = nc.declare_dram_parameter("stt", [128, 8, B], BF16, isOutput=False)
    ones_in = nc.declare_dram_parameter("ones", [1, B], BF16, isOutput=False)
    whd_in = nc.declare_dram_parameter("whd", [128, 8, 4, 128], BF16, isOutput=False)
    b1_in = nc.declare_dram_parameter("b1c", [1, 4, 128], BF16, isOutput=False)
    u_in = nc.declare_dram_parameter("u", [128, VTILES, 4, 128], BF16, isOutput=False)
    b2_in = nc.declare_dram_parameter("b2c", [128, VTILES], F32, isOutput=False)
    out = nc.declare_dram_parameter("logitsT", [128, VTILES, B], BF16, isOutput=True)

    VCH = 7  # vt tiles per DMA chunk (49 = 7*7)
    with tile.TileContext(nc) as tc:
        with (
            tc.tile_pool(name="const", bufs=1) as cpool,
            tc.tile_pool(name="io", bufs=4) as io,
            tc.tile_pool(name="ob", bufs=2) as obp,
            tc.tile_pool(name="psum", bufs=4, space="PSUM") as psp,
        ):
            stt = cpool.tile([128, 8, B], BF16)
            nc.sync.dma_start(stt[:], stt_in[:])
            ones = cpool.tile([1, B], BF16)
            nc.sync.dma_start(ones[:], ones_in[:])
            whd = cpool.tile([128, 8, 4, 128], BF16)
            nc.sync.dma_start(whd[:], whd_in[:])
            b1c = cpool.tile([1, 4, 128], BF16)
            nc.sync.dma_start(b1c[:], b1_in[:])
            b2c = cpool.tile([128, VTILES], F32)
            nc.sync.dma_start(b2c[:], b2_in[:])
            u_sb = cpool.tile([128, VTILES, 4, 128], BF16)
            for v0 in range(0, VTILES, VCH):
                nc.sync.dma_start(u_sb[:, v0:v0 + VCH], u_in[:, v0:v0 + VCH])

            # h = relu(states @ W_head + b1), gate-major: out [128 hfeat, B]
            hsb = io.tile([128, 4, B], BF16, tag="h")
            for oc in range(4):
                ps = psp.tile([128, B], F32, tag="h", name="psh")
                nc.tensor.matmul(ps[:], b1c[:, oc, :], ones[:],
                                 start=True, stop=False)
                for k in range(8):
                    nc.tensor.matmul(ps[:], whd[:, k, oc, :], stt[:, k, :],
                                     start=False, stop=(k == 7))
                nc.scalar.activation(hsb[:, oc, :], ps[:], ACT.Relu)
            # logits tiles; stage VCH output tiles per store DMA
            obuf = None
            for vt in range(VTILES):
                if vt % VCH == 0:
                    obuf = obp.tile([128, VCH, B], BF16, tag="ob", name="ob")
                psl = psp.tile([128, B], F32, tag="l", name="psl")
                for k in range(4):
                    nc.tensor.matmul(psl[:], u_sb[:, vt, k, :], hsb[:, k, :],
                                     start=(k == 0), stop=(k == 3))
                nc.scalar.activation(obuf[:, vt % VCH, :], psl[:], ACT.Identity,
                                     bias=b2c[:, vt:vt + 1])
                if vt % VCH == VCH - 1:
                    v0 = vt - (VCH - 1)
                    nc.sync.dma_start(out[:, v0:v0 + VCH, :], obuf[:])
    _split_sync_waits(nc)
    _cache["head"] = nc
    return nc


def _prep_cell_weights(Wx, Wh, b):
    """Host-side: build exact-SBUF-layout weight arrays (gate-major).

    Gate order in the 2048 columns is [j, i, f, o] per feature chunk layout
    [gate, fc, 128]; here we keep reference order [i, j, f, o] remapped to
    device gates (0=j tanh, 1=i, 2=f, 3=o)."""
    Wx = np.asarray(Wx, np.float32)
    Wh = np.asarray(Wh, np.float32)
    b = np.asarray(b, np.float32).copy()
    b[2 * D:3 * D] += 1.0  # forget_bias
    kin = Wx.shape[0]
    # reference column order: i, j, f, o; device order j, i, f, o
    perm = np.concatenate([np.arange(D, 2 * D), np.arange(0, D),
                           np.arange(2 * D, 3 * D), np.arange(3 * D, 4 * D)])
    Wxp = Wx[:, perm]    # [kin, 2048] cols = (gate, fc, m)
    Whp = Wh[:, perm]
    bp = b[perm]
    # psign per device gate: j:0, i:-1, f:+1, o:-1
    psign = np.concatenate([np.zeros(D, np.float32), -np.ones(D, np.float32),
                            np.ones(D, np.float32), -np.ones(D, np.float32)])
    # wx: [128, KC, 16, 128]: wx[p, k, oc, m] = Wxp[k*128+p, oc*128+m]
    KC = kin // 128
    wx = Wxp.reshape(KC, 128, 16, 128).transpose(1, 0, 2, 3)
    # wh: [128, 4, 16, 128]
    wh = Whp.reshape(4, 128, 16, 128).transpose(1, 0, 2, 3)
    # wb: [2, 16, 128]: row0 bias, row1 BIG*psign
    wb = np.stack([bp.reshape(16, 128), BIG * psign.reshape(16, 128)])
    return (np.ascontiguousarray(wx).astype(NPBF),
            np.ascontiguousarray(wh).astype(NPBF),
            np.ascontiguousarray(wb).astype(NPBF))


def layer_inputs(xt_by_core, m, wprep2):
    """Per-core input maps for one layer launch.
    xt_by_core: list of 8 arrays [128, KC, T, 64] (already direction-reversed);
    m: [B, T] validity mask; wprep2: (fwd, bwd) tuples from _prep_cell_weights."""
    maps = []
    for cc in range(NC):
        q, rev = cc % 4, cc >= 4
        mq = m[q * BSH:(q + 1) * BSH]          # [64, T]
        if rev:
            mq = mq[:, ::-1]
        m2 = np.empty((2, T, BSH), np.float32)
        m2[0] = 1.0
        m2[1] = (1.0 - mq).T
        wx, wh, wb = wprep2[1 if rev else 0]
        maps.append({"xt": xt_by_core[cc], "m2": m2.astype(NPBF),
                     "wx": wx, "wb": wb, "wh": wh})
    return maps


def _run(nc, in_maps, trace=False):
    return run_bass_kernel_spmd(nc, in_maps, core_ids=list(range(NC)), trace=trace)


last_exec_ns = {}


def kernel(tokens, lengths, embedding, Wx_f0, Wh_f0, b_f0, Wx_b0, Wh_b0, b_b0,
           Wx_f1, Wh_f1, b_f1, Wx_b1, Wh_b1, b_b1, W_head, b1, U, b2,
           trace=False):
    tokens = np.asarray(tokens)
    lengths = np.asarray(lengths)
    embedding = np.asarray(embedding, np.float32)

    if "wprep" not in _cache:
        cells = {}
        for nm, (wx, wh, bb) in (("f0", (Wx_f0, Wh_f0, b_f0)), ("b0", (Wx_b0, Wh_b0, b_b0)),
                                 ("f1", (Wx_f1, Wh_f1, b_f1)), ("b1", (Wx_b1, Wh_b1, b_b1))):
            cells[nm] = _prep_cell_weights(wx, wh, bb)
        _cache["wprep"] = cells
    cells = _cache["wprep"]

    m = (np.arange(T)[None, :] < lengths[:, None]).astype(np.float32)  # [B, T]

    # ---- layer 0 ----
    x0 = embedding[tokens]                       # [B, T, E] f32
    xt0 = []
    for cc in range(NC):
        q, rev = cc % 4, cc >= 4
        xq = x0[q * BSH:(q + 1) * BSH]           # [64, T, 128]
        if rev:
            xq = xq[:, ::-1]
        # [128, 1, T, 64]
        xt0.append(np.ascontiguousarray(
            xq.transpose(2, 1, 0)[:, None]).astype(NPBF))
    r0 = _run(get_layer_nc(0), layer_inputs(xt0, m, (cells["f0"], cells["b0"])),
              trace=trace)
    if r0.exec_time_ns:
        last_exec_ns["layer0"] = r0.exec_time_ns

    # y per core: [T, 128, 4, 64] -> feature f = fc*128+p of quarter q
    # build layer1 xt [128, 8, T, 64]: kk 0..3 = fw chunks, 4..7 = bw chunks
    yf = [np.asarray(r0.results[q]["y"], np.float32) for q in range(4)]
    yb = [np.asarray(r0.results[4 + q]["y"], np.float32)[::-1] for q in range(4)]
    # yb un-reversed to original time
    xt1 = []
    for cc in range(NC):
        q, rev = cc % 4, cc >= 4
        a = np.empty((128, 8, T, BSH), np.float32)
        f, bwd = yf[q], yb[q]
        if rev:
            f, bwd = f[::-1], bwd[::-1]
        # y layout [T, p, fc, b] -> xt [p, kk, t, b]
        a[:, 0:4] = f.transpose(1, 2, 0, 3)
        a[:, 4:8] = bwd.transpose(1, 2, 0, 3)
        xt1.append(np.ascontiguousarray(a).astype(NPBF))
    r1 = _run(get_layer_nc(1), layer_inputs(xt1, m, (cells["f1"], cells["b1"])),
              trace=trace)
    if r1.exec_time_ns:
        last_exec_ns["layer1"] = r1.exec_time_ns

    # states: feature-major [128, 4, 64] f32 per core -> sttT [1024, B]
    sttT = np.empty((2 * D, B), np.float32)
    for q in range(4):
        bsl = slice(q * BSH, (q + 1) * BSH)
        cf = r1.results[q]["states"]             # [128, 4, 64]
        cb = r1.results[4 + q]["states"]
        sttT[0:D, bsl] = cf.transpose(1, 0, 2).reshape(D, BSH)
        sttT[D:2 * D, bsl] = cb.transpose(1, 0, 2).reshape(D, BSH)

    # ---- head ----
    if "headprep" not in _cache:
        W_head = np.asarray(W_head, np.float32)
        b1 = np.asarray(b1, np.float32)
        U = np.asarray(U, np.float32)
        b2 = np.asarray(b2, np.float32)
        whd = W_head.reshape(8, 128, 4, 128).transpose(1, 0, 2, 3)
        b1c = b1.reshape(1, 4, 128)
        vs = V // NC
        u_by_core, b2_by_core = [], []
        for cc in range(NC):
            u_pad = np.zeros((D, VSH), np.float32)
            u_pad[:, 0:vs] = U[:, cc * vs:(cc + 1) * vs]
            b2_pad = np.zeros(VSH, np.float32)
            b2_pad[0:vs] = b2[cc * vs:(cc + 1) * vs]
            # u: [128, VTILES, 4, 128]: u[p, vt, kk, m] = U[kk*128+p, vt*128+m]
            uu = u_pad.reshape(4, 128, VTILES, 128).transpose(1, 2, 0, 3)
            u_by_core.append(np.ascontiguousarray(uu).astype(NPBF))
            b2_by_core.append(np.ascontiguousarray(
                b2_pad.reshape(VTILES, 128).T))
        _cache["headprep"] = (
            np.ascontiguousarray(whd).astype(NPBF),
            np.ascontiguousarray(b1c).astype(NPBF),
            u_by_core, b2_by_core,
            np.ones((1, B), NPBF))
    whd, b1c, u_by_core, b2_by_core, ones = _cache["headprep"]

    stt = np.ascontiguousarray(
        sttT.reshape(8, 128, B).transpose(1, 0, 2)).astype(NPBF)
    in_maps2 = [{"stt": stt, "ones": ones, "whd": whd, "b1c": b1c,
                 "u": u_by_core[cc], "b2c": b2_by_core[cc]} for cc in range(NC)]
    r2 = _run(get_head_nc(), in_maps2, trace=trace)
    if r2.exec_time_ns:
        last_exec_ns["head"] = r2.exec_time_ns

    vs = V // NC
    logits = np.empty((B, V), np.float32)
    for cc in range(NC):
        lt = np.asarray(r2.results[cc]["logitsT"], np.float32)  # [128, 49, 256]
        lc = lt.transpose(1, 0, 2).reshape(VSH, B).T
        logits[:, cc * vs:(cc + 1) * vs] = lc[:, 0:vs]
    return logits


# revision 18
# speedup vs baseline: 1.0683x; 1.0683x over previous
"""Trainium2 Bass kernel for nn_BidirLSTMModel (2-layer bidirectional LSTM + vocab head).

Sharding: each LSTM layer runs as one 8-core SPMD launch sharded by
(direction x batch-quarter): cores 0-3 = forward cells on batch quarters 0-3,
cores 4-7 = backward cells (inputs time-reversed on the host, so every core
runs the identical forward-scan program) => 64 batch rows per core. The output
head is a third launch sharded by vocab. The host moves the small intermediate
tensors between launches (free: only device launches count toward HW time).

Device layout is GATE-MAJOR throughout: weights are the 128x128 stationary
operands (FWL keeps LDWEIGHTS fully hidden; measured 38 ns per N=64 matmul),
h/c/gates all live as [feature-part, chunk, batch] so the recurrence needs no
transpose anywhere. Per 2-step PSUM group and gate g (psum tile [128,2,4,64]):
  - one [bias; +/-BIG*(1-mask)] rank-2 matmul per (gate, feature-chunk) seeds
    bias + length-mask penalties (saturating i/f/o sigmoids reproduces
    dynamic_rnn's masking exactly, as in the original reference trick),
  - xg matmuls stream 2 timesteps per instruction (N=128, 89% PE eff),
  - 4 recurrent Wh matmuls per (step, gate, fchunk) at N=64,
  - scalar engine applies tanh/sigmoid straight off PSUM slices,
  - vector engine does the c/h elementwise chain; h feeds the next step's
    matmuls directly as the moving operand.
"""

import numpy as np
import ml_dtypes

import concourse.bass as bass
import concourse.mybir as mybir
import concourse.tile as tile
from concourse.bass_utils import run_bass_kernel_spmd


def _split_sync_waits(nc, max_waits=1):
    """This walrus build accepts at most one sync-wait per instruction; hoist
    extra waits onto same-engine NoOps placed immediately before (same queue,
    program order => identical wait-all semantics)."""
    n = 0
    for f in nc.m.functions:
        for bb in f.blocks:
            out = []
            for ins in bb.instructions:
                si = ins.sync_info
                if si is not None and si.on_wait and len(si.on_wait) > max_waits:
                    waits = list(si.on_wait)
                    for w in waits[:-max_waits]:
                        nop = mybir.InstNoOp(name=f"{ins.name}-ws{n}", ins=[], outs=[])
                        n += 1
                        nop.engine = ins.engine
                        nop.sync_info = mybir.SyncInfo(on_wait=[w], on_update=[])
                        out.append(nop)
                    si.on_wait = waits[-max_waits:]
                out.append(ins)
            bb.instructions[:] = out


BF16 = mybir.dt.bfloat16
F32 = mybir.dt.float32
NPBF = ml_dtypes.bfloat16

V, E, D, B, T = 50000, 128, 512, 256, 128
NC = 8
BSH = B // (NC // 2)   # 64 batch rows per core
BIG = 30.0
VSH = 6272             # padded vocab shard (49*128); 8*6250 = 50000
VTILES = VSH // 128    # 49
TB = 16                # xt streaming block (timesteps)

ACT = mybir.ActivationFunctionType
OP = mybir.AluOpType

_cache = {}


def _layer_program(nc, tc, pools, KC, xt_in, m2_in, wx_in, wb_in, wh_in,
                   y_out, states_out):
    """One direction of one layer, 64 batch rows, gate-major."""
    (cpool, xs, sv, cst, psp) = pools

    wh_sb = cpool.tile([128, 4, 16, 128], BF16)
    nc.sync.dma_start(wh_sb[:], wh_in[:])
    wx_sb = cpool.tile([128, KC, 16, 128], BF16)
    nc.sync.dma_start(wx_sb[:], wx_in[:])
    wb_sb = cpool.tile([2, 16, 128], BF16)
    nc.sync.dma_start(wb_sb[:], wb_in[:])
    m2_sb = cpool.tile([2, T, BSH], BF16)
    nc.sync.dma_start(m2_sb[:], m2_in[:])

    # xt: layer0's fits in SBUF whole (2 MB) — load it all so the scan never
    # waits on input DMA; layer1's (16 MB) streams in TB-step blocks.
    LEAD = 2
    xblks = {}

    if KC == 1:
        xall = cpool.tile([128, 1, T, BSH], BF16)
        for qq in range(4):
            nc.sync.dma_start(xall[:, :, qq * (T // 4):(qq + 1) * (T // 4), :],
                              xt_in[:, :, qq * (T // 4):(qq + 1) * (T // 4), :])

        def load_xblk(bi):
            if bi * TB < T:
                xblks[bi] = xall[:, :, bi * TB:(bi + 1) * TB, :]
    else:
        def load_xblk(bi):
            if bi >= T // TB:
                return
            tl = xs.tile([128, KC, TB, BSH], BF16, tag="xb", name="xb")
            for k in range(KC):
                nc.sync.dma_start(tl[:, k], xt_in[:, k, bi * TB:(bi + 1) * TB, :])
            xblks[bi] = tl

    for bi in range(LEAD):
        load_xblk(bi)

    h = sv.tile([128, 4, BSH], BF16, tag="h")
    nc.vector.memset(h[:], 0.0)
    c = cst.tile([128, 4, BSH], F32, tag="c")
    nc.vector.memset(c[:], 0.0)

    GS = 2  # steps per PSUM group
    # device gate ids: 0=j(tanh), 1=i, 2=f, 3=o. PE phases run f,i,j,o so
    # sig_f lands first (the c-path v=f*c starts on GpSimd while i/j matmul),
    # and o last (its sigmoid + h-mult are the unavoidable step tail).
    GORDER = (2, 1, 0, 3)
    for g in range(T // GS):
        s0 = GS * g
        if s0 % TB == 0:
            load_xblk(s0 // TB + LEAD)
        xb = xblks[s0 // TB]
        xo = s0 % TB

        # psum gate tiles: [128, fc, step, batch] so every matmul dst is a
        # contiguous slice (matmul outputs cannot be strided APs).
        # start=True marks the whole 2KB PSUM bank pending-zero, so only the
        # tile's first matmul carries it; later slice-writers land on
        # still-pending bytes and overwrite cleanly.
        pt = {gt: psp.tile([128, 4, GS, BSH], F32, tag=f"g{gt}", name=f"g{gt}")
              for gt in GORDER}
        for gt in GORDER:
            # bias + mask-penalty seed matmuls
            for fc in range(4):
                nc.tensor.matmul(pt[gt][:, fc],
                                 wb_sb[:, 4 * gt + fc, :],
                                 m2_sb[:, s0:s0 + GS, :],
                                 start=(fc == 0), stop=False,
                                 skip_group_check=True)
            # input projection, GS steps per matmul
            for fc in range(4):
                for k in range(KC):
                    nc.tensor.matmul(pt[gt][:, fc],
                                     wx_sb[:, k, 4 * gt + fc, :],
                                     xb[:, k, xo:xo + GS, :],
                                     start=False, stop=False,
                                     skip_group_check=True)
        # GS sequential steps
        for ss in range(GS):
            s = s0 + ss
            sig = {}
            u = v = c2 = tcn = None
            for gt in GORDER:
                for fc in range(4):
                    for kk in range(4):
                        nc.tensor.matmul(pt[gt][:, fc, ss, :],
                                         wh_sb[:, kk, 4 * gt + fc, :],
                                         h[:, kk, :],
                                         start=False, stop=(kk == 3),
                                         skip_group_check=True)
                g_t = sv.tile([128, 4, BSH], BF16, tag=f"s{gt}", name=f"s{gt}")
                nc.scalar.activation(g_t[:], pt[gt][:, :, ss, :],
                                     ACT.Tanh if gt == 0 else ACT.Sigmoid)
                sig[gt] = g_t
                if gt == 2:
                    v = sv.tile([128, 4, BSH], F32, tag="v")
                    nc.gpsimd.tensor_tensor(v[:], sig[2][:], c[:], OP.mult)
                elif gt == 0:
                    u = sv.tile([128, 4, BSH], BF16, tag="u")
                    nc.vector.tensor_tensor(u[:], sig[1][:], sig[0][:], OP.mult)
                    c2 = cst.tile([128, 4, BSH], F32, tag="c")
                    nc.vector.tensor_tensor(c2[:], v[:], u[:], OP.add)
            # tanh(c2) is emitted after sig_o so the in-order scalar queue
            # doesn't block o's sigmoid behind the c2 dependency
            tcn = sv.tile([128, 4, BSH], BF16, tag="tcn")
            nc.scalar.activation(tcn[:], c2[:], ACT.Tanh)
            h = sv.tile([128, 4, BSH], BF16, tag="h")
            nc.vector.tensor_tensor(h[:], sig[3][:], tcn[:], OP.mult)
            nc.sync.dma_start(y_out[s], h[:])
            c = c2

    nc.sync.dma_start(states_out[:], c[:])


def get_layer_nc(layer):
    key = f"layer{layer}"
    if key in _cache:
        return _cache[key]
    KC = 1 if layer == 0 else 8
    nc = bass.Bass()
    xt_in = nc.declare_dram_parameter("xt", [128, KC, T, BSH], BF16, isOutput=False)
    m2_in = nc.declare_dram_parameter("m2", [2, T, BSH], BF16, isOutput=False)
    wx_in = nc.declare_dram_parameter("wx", [128, KC, 16, 128], BF16, isOutput=False)
    wb_in = nc.declare_dram_parameter("wb", [2, 16, 128], BF16, isOutput=False)
    wh_in = nc.declare_dram_parameter("wh", [128, 4, 16, 128], BF16, isOutput=False)
    y_out = nc.declare_dram_parameter("y", [T, 128, 4, BSH], BF16, isOutput=True)
    states_out = nc.declare_dram_parameter("states", [128, 4, BSH], F32, isOutput=True)

    with tile.TileContext(nc) as tc:
        with (
            tc.tile_pool(name="const", bufs=1) as cpool,
            tc.tile_pool(name="xs", bufs=3) as xs,
            tc.tile_pool(name="sv", bufs=3) as sv,
            tc.tile_pool(name="cst", bufs=2) as cst,
            tc.tile_pool(name="psum", bufs=1, space="PSUM") as psp,
        ):
            pools = (cpool, xs, sv, cst, psp)
            _layer_program(nc, tc, pools, KC, xt_in, m2_in, wx_in, wb_in,
                           wh_in, y_out, states_out)
    _split_sync_waits(nc)
    _cache[key] = nc
    return nc


def get_head_nc():
    if "head" in _cache:
        return _cache["head"]
    nc = bass.Bass()
    stt_in = nc.declare_dram_parameter("stt", [128, 8, B], BF16, isOutput=False)
    ones_in = nc.declare_dram_parameter("ones", [1, B], BF16, isOutput=False)
    whd_in = nc.declare_dram_parameter("whd", [128, 8, 4, 128], BF16, isOutput=False)
    b1_in = nc.declare_dram_parameter("b1c", [1, 4, 128], BF16, isOutput=False)
    u_in = nc.declare_dram_parameter("u", [128, VTILES, 4, 128], BF16, isOutput=False)
    b2_in = nc.declare_dram_parameter("b2c", [128, VTILES], F32, isOutput=False)
    out = nc.declare_dram_parameter("logitsT", [128, VTILES, B], BF16, isOutput=True)

    VCH = 7  # vt tiles per DMA chunk (49 = 7*7)
    with tile.TileContext(nc) as tc:
        with (
            tc.tile_pool(name="const", bufs=1) as cpool,
            tc.tile_pool(name="io", bufs=4) as io,
            tc.tile_pool(name="ob", bufs=2) as obp,
            tc.tile_pool(name="psum", bufs=4, space="PSUM") as psp,
        ):
            stt = cpool.tile([128, 8, B], BF16)
            nc.sync.dma_start(stt[:], stt_in[:])
            ones = cpool.tile([1, B], BF16)
            nc.sync.dma_start(ones[:], ones_in[:])
            whd = cpool.tile([128, 8, 4, 128], BF16)
            nc.sync.dma_start(whd[:], whd_in[:])
            b1c = cpool.tile([1, 4, 128], BF16)
            nc.sync.dma_start(b1c[:], b1_in[:])
            b2c = cpool.tile([128, VTILES], F32)
            nc.sync.dma_start(b2c[:], b2_in[:])
            u_sb = cpool.tile([128, VTILES, 4, 128], BF16)
            for v0 in range(0, VTILES, VCH):
                nc.sync.dma_start(u_sb[:, v0:v0 + VCH], u_in[:, v0:v0 + VCH])

            # h = relu(states @ W_head + b1), gate-major: out [128 hfeat, B]
            hsb = io.tile([128, 4, B], BF16, tag="h")
            for oc in range(4):
                ps = psp.tile([128, B], F32, tag="h", name="psh")
                nc.tensor.matmul(ps[:], b1c[:, oc, :], ones[:],
                                 start=True, stop=False)
                for k in range(8):
                    nc.tensor.matmul(ps[:], whd[:, k, oc, :], stt[:, k, :],
                                     start=False, stop=(k == 7))
                nc.scalar.activation(hsb[:, oc, :], ps[:], ACT.Relu)
            # logits tiles; stage VCH output tiles per store DMA
            obuf = None
            for vt in range(VTILES):
                if vt % VCH == 0:
                    obuf = obp.tile([128, VCH, B], BF16, tag="ob", name="ob")
                psl = psp.tile([128, B], F32, tag="l", name="psl")
                for k in range(4):
                    nc.tensor.matmul(psl[:], u_sb[:, vt, k, :], hsb[:, k, :],
                                     start=(k == 0), stop=(k == 3))
                nc.scalar.activation(obuf[:, vt % VCH, :], psl[:], ACT.Identity,
                                     bias=b2c[:, vt:vt + 1])
                if vt % VCH == VCH - 1:
                    v0 = vt - (VCH - 1)
                    nc.sync.dma_start(out[:, v0:v0 + VCH, :], obuf[:])
    _split_sync_waits(nc)
    _cache["head"] = nc
    return nc


def _prep_cell_weights(Wx, Wh, b):
    """Host-side: build exact-SBUF-layout weight arrays (gate-major).

    Gate order in the 2048 columns is [j, i, f, o] per feature chunk layout
    [gate, fc, 128]; here we keep reference order [i, j, f, o] remapped to
    device gates (0=j tanh, 1=i, 2=f, 3=o)."""
    Wx = np.asarray(Wx, np.float32)
    Wh = np.asarray(Wh, np.float32)
    b = np.asarray(b, np.float32).copy()
    b[2 * D:3 * D] += 1.0  # forget_bias
    kin = Wx.shape[0]
    # reference column order: i, j, f, o; device order j, i, f, o
    perm = np.concatenate([np.arange(D, 2 * D), np.arange(0, D),
                           np.arange(2 * D, 3 * D), np.arange(3 * D, 4 * D)])
    Wxp = Wx[:, perm]    # [kin, 2048] cols = (gate, fc, m)
    Whp = Wh[:, perm]
    bp = b[perm]
    # psign per device gate: j:0, i:-1, f:+1, o:-1
    psign = np.concatenate([np.zeros(D, np.float32), -np.ones(D, np.float32),
                            np.ones(D, np.float32), -np.ones(D, np.float32)])
    # wx: [128, KC, 16, 128]: wx[p, k, oc, m] = Wxp[k*128+p, oc*128+m]
    KC = kin // 128
    wx = Wxp.reshape(KC, 128, 16, 128).transpose(1, 0, 2, 3)
    # wh: [128, 4, 16, 128]
    wh = Whp.reshape(4, 128, 16, 128).transpose(1, 0, 2, 3)
    # wb: [2, 16, 128]: row0 bias, row1 BIG*psign
    wb = np.stack([bp.reshape(16, 128), BIG * psign.reshape(16, 128)])
    return (np.ascontiguousarray(wx).astype(NPBF),
            np.ascontiguousarray(wh).astype(NPBF),
            np.ascontiguousarray(wb).astype(NPBF))


def layer_inputs(xt_by_core, m, wprep2):
    """Per-core input maps for one layer launch.
    xt_by_core: list of 8 arrays [128, KC, T, 64] (already direction-reversed);
    m: [B, T] validity mask; wprep2: (fwd, bwd) tuples from _prep_cell_weights."""
    maps = []
    for cc in range(NC):
        q, rev = cc % 4, cc >= 4
        mq = m[q * BSH:(q + 1) * BSH]          # [64, T]
        if rev:
            mq = mq[:, ::-1]
        m2 = np.empty((2, T, BSH), np.float32)
        m2[0] = 1.0
        m2[1] = (1.0 - mq).T
        wx, wh, wb = wprep2[1 if rev else 0]
        maps.append({"xt": xt_by_core[cc], "m2": m2.astype(NPBF),
                     "wx": wx, "wb": wb, "wh": wh})
    return maps


def _run(nc, in_maps, trace=False):
    return run_bass_kernel_spmd(nc, in_maps, core_ids=list(range(NC)), trace=trace)


last_exec_ns = {}


def kernel(tokens, lengths, embedding, Wx_f0, Wh_f0, b_f0, Wx_b0, Wh_b0, b_b0,
           Wx_f1, Wh_f1, b_f1, Wx_b1, Wh_b1, b_b1, W_head, b1, U, b2,
           trace=False):
    tokens = np.asarray(tokens)
    lengths = np.asarray(lengths)
    embedding = np.asarray(embedding, np.float32)

    if "wprep" not in _cache:
        cells = {}
        for nm, (wx, wh, bb) in (("f0", (Wx_f0, Wh_f0, b_f0)), ("b0", (Wx_b0, Wh_b0, b_b0)),
                                 ("f1", (Wx_f1, Wh_f1, b_f1)), ("b1", (Wx_b1, Wh_b1, b_b1))):
            cells[nm] = _prep_cell_weights(wx, wh, bb)
        _cache["wprep"] = cells
    cells = _cache["wprep"]

    m = (np.arange(T)[None, :] < lengths[:, None]).astype(np.float32)  # [B, T]

    # ---- layer 0 ----
    x0 = embedding[tokens]                       # [B, T, E] f32
    xt0 = []
    for cc in range(NC):
        q, rev = cc % 4, cc >= 4
        xq = x0[q * BSH:(q + 1) * BSH]           # [64, T, 128]
        if rev:
            xq = xq[:, ::-1]
        # [128, 1, T, 64]
        xt0.append(np.ascontiguousarray(
            xq.transpose(2, 1, 0)[:, None]).astype(NPBF))
    r0 = _run(get_layer_nc(0), layer_inputs(xt0, m, (cells["f0"], cells["b0"])),
              trace=trace)
    if r0.exec_time_ns:
        last_exec_ns["layer0"] = r0.exec_time_ns

    # y per core: [T, 128, 4, 64] -> feature f = fc*128+p of quarter q
    # build layer1 xt [128, 8, T, 64]: kk 0..3 = fw chunks, 4..7 = bw chunks
    yf = [np.asarray(r0.results[q]["y"], np.float32) for q in range(4)]
    yb = [np.asarray(r0.results[4 + q]["y"], np.float32)[::-1] for q in range(4)]
    # yb un-reversed to original time
    xt1 = []
    for cc in range(NC):
        q, rev = cc % 4, cc >= 4
        a = np.empty((128, 8, T, BSH), np.float32)
        f, bwd = yf[q], yb[q]
        if rev:
            f, bwd = f[::-1], bwd[::-1]
        # y layout [T, p, fc, b] -> xt [p, kk, t, b]
        a[:, 0:4] = f.transpose(1, 2, 0, 3)
        a[:, 4:8] = bwd.transpose(1, 2, 0, 3)
        xt1.append(np.ascontiguousarray(a).astype(NPBF))
    r1 = _run(get_layer_nc(1), layer_inputs(xt1, m, (cells["f1"], cells["b1"])),
              trace=trace)
    if r1.exec_time_ns:
        last_exec_ns["layer1"] = r1.exec_time_ns

    # states: feature-major [128, 4, 64] f32 per core -> sttT [1024, B]
    sttT = np.empty((2 * D, B), np.float32)
    for q in range(4):
        bsl = slice(q * BSH, (q + 1) * BSH)
        cf = r1.results[q]["states"]             # [128, 4, 64]
        cb = r1.results[4 + q]["states"]
        sttT[0:D, bsl] = cf.transpose(1, 0, 2).reshape(D, BSH)
        sttT[D:2 * D, bsl] = cb.transpose(1, 0, 2).reshape(D, BSH)

    # ---- head ----
    if "headprep" not in _cache:
        W_head = np.asarray(W_head, np.float32)
        b1 = np.asarray(b1, np.float32)
        U = np.asarray(U, np.float32)
        b2 = np.asarray(b2, np.float32)
        whd = W_head.reshape(8, 128, 4, 128).transpose(1, 0, 2, 3)
        b1c = b1.reshape(1, 4, 128)
        vs = V // NC
        u_by_core, b2_by_core = [], []
        for cc in range(NC):
            u_pad = np.zeros((D, VSH), np.float32)
            u_pad[:, 0:vs] = U[:, cc * vs:(cc + 1) * vs]
            b2_pad = np.zeros(VSH, np.float32)
            b2_pad[0:vs] = b2[cc * vs:(cc + 1) * vs]
            # u: [128, VTILES, 4, 128]: u[p, vt, kk, m] = U[kk*128+p, vt*128+m]
            uu = u_pad.reshape(4, 128, VTILES, 128).transpose(1, 2, 0, 3)
            u_by_core.append(np.ascontiguousarray(uu).astype(NPBF))
            b2_by_core.append(np.ascontiguousarray(
                b2_pad.reshape(VTILES, 128).T))
        _cache["headprep"] = (
            np.ascontiguousarray(whd).astype(NPBF),
            np.ascontiguousarray(b1c).astype(NPBF),
            u_by_core, b2_by_core,
            np.ones((1, B), NPBF))
    whd, b1c, u_by_core, b2_by_core, ones = _cache["headprep"]

    stt = np.ascontiguousarray(
        sttT.reshape(8, 128, B).transpose(1, 0, 2)).astype(NPBF)
    in_maps2 = [{"stt": stt, "ones": ones, "whd": whd, "b1c": b1c,
                 "u": u_by_core[cc], "b2c": b2_by_core[cc]} for cc in range(NC)]
    r2 = _run(get_head_nc(), in_maps2, trace=trace)
    if r2.exec_time_ns:
        last_exec_ns["head"] = r2.exec_time_ns

    vs = V // NC
    logits = np.empty((B, V), np.float32)
    for cc in range(NC):
        lt = np.asarray(r2.results[cc]["logitsT"], np.float32)  # [128, 49, 256]
        lc = lt.transpose(1, 0, 2).reshape(VSH, B).T
        logits[:, cc * vs:(cc + 1) * vs] = lc[:, 0:vs]
    return logits
